# revision 53
# baseline (speedup 1.0000x reference)
"""Trainium2 Bass kernel: 16-head causal MHA (B=2, S=2048, hidden=1024).

Sharding (data + head parallel over 8 cores): core c handles batch c//4
and heads [4*(c%4), 4*(c%4)+4). Each core computes its q/k/v projections,
causal attention for its 4 heads, and a partial o-projection restricted to
its head columns. The host sums the 4 partials per batch (the post-o_proj
all-reduce, done host-side during gather) and adds the exactly-linear bias
terms (bv @ wo.T + bo). bq/bk are applied on device via rank-1 bias
matmuls when nonzero.

Design (all matmuls bf16 at full PE rate; 1.48x over the fp32r version,
189315 -> 127752 ns):
  - the host passes x and all weights pre-converted to bf16, so batched
    strided DMA loads land directly in matmul-ready SBUF tiles (no
    fp32r staging converts, and half the load traffic);
  - scores are computed transposed, scoresT[t, s] = kT-slice.T @ qT-slice;
    softmax normalization sums arrive free by augmenting v with a ones
    column (row 64 of the PV output is the denominator), and exp skips
    max-subtraction (scores are O(1) for this problem);
  - diagonal trims: the (r2,r3) score pair computes/exps only columns
    [256:512) (r3 just [384:512), with its own exact-range exp); PV skips
    below-diagonal columns per tile (c0 = 128*r), so masking is just 4
    triangular muls per head-chunk, no zero fills;
  - the first projection pass streams per contraction tile against the
    arriving x DMAs so the PE starts at ~4.5us; later projection passes
    and one-s-chunk-deferred o_proj chunks fill PE bubbles between heads;
  - attention is software-pipelined at head granularity: QK+exp of head
    h+1 issues before PV of head h and normalization runs two heads
    behind, so the scalar engine's exp stream never starves the PE;
  - the final s-chunk's o_proj pre-issues its dti0 half-matmuls into the
    idle score/pv psum banks during the last head's exp window, and the
    last softmax normalization runs as two parallel half-width chains;
  - engine budget: PE ~117us (saturated), ACT exp-only ~84us, DVE all
    PSUM-adjacent copies/masks/norms ~60us, GPSIMD the SBUF-only
    partition broadcasts. GPSIMD cannot access PSUM (real BIR verifier
    rule), and no instruction may read two PSUM operands.
"""

import numpy as np
import ml_dtypes

import concourse.bass as bass
import concourse.mybir as mybir
import concourse.tile as tile
from concourse import bacc
from concourse.bass_utils import run_bass_kernel_spmd

B, S, HID = 2, 2048, 1024
NH, HD = 16, 64
N_CORES = 8
HPC = 4            # heads per core
DPC = HPC * HD     # 256 head-dims per core
SC = 512           # s-chunk (matmul free dim)
NSC = S // SC      # 4
TT = 128           # t-tile (partitions)
NTT = S // TT      # 16
NKT = HID // 128   # 8 contraction tiles for the projections

F32 = mybir.dt.float32
BF16 = mybir.dt.bfloat16
EXP = mybir.ActivationFunctionType.Exp
COPY = mybir.ActivationFunctionType.Copy
BF = ml_dtypes.bfloat16


def _build(causal: bool, has_bias: bool = False):
    nc = bacc.Bacc(
        "TRN2",
        target_bir_lowering=False,
        debug=False,
        enable_asserts=False,
        num_devices=N_CORES,
    )
    xT = nc.dram_tensor("xT", [HID, S], BF16, kind="ExternalInput").ap()
    wqT = nc.dram_tensor("wqT", [HID, DPC], BF16, kind="ExternalInput").ap()
    wkT = nc.dram_tensor("wkT", [HID, DPC], BF16, kind="ExternalInput").ap()
    wvT = nc.dram_tensor("wvT", [HID, DPC], BF16, kind="ExternalInput").ap()
    woT = nc.dram_tensor("woT", [DPC, HID], BF16, kind="ExternalInput").ap()
    mskd = nc.dram_tensor("mask_tri", [TT, TT], BF16, kind="ExternalInput").ap()
    if has_bias:
        bqr = nc.dram_tensor("bq_r", [1, DPC], BF16, kind="ExternalInput").ap()
        bkr = nc.dram_tensor("bk_r", [1, DPC], BF16, kind="ExternalInput").ap()
    outT = nc.dram_tensor("outT", [HID, S], BF16, kind="ExternalOutput").ap()

    scale = float(1.0 / np.sqrt(HD))

    ctx_lp = nc.allow_low_precision(reason="bf16 matmul pipeline (deliberate)")
    ctx_lp.__enter__()
    with tile.TileContext(nc) as tc:
        with (
            tc.tile_pool(name="persist", bufs=1) as pp,
            tc.tile_pool(name="expbuf", bufs=8) as e_pool,
            tc.tile_pool(name="attn", bufs=4) as attn_pool,
            tc.tile_pool(name="osb", bufs=6) as o_pool,
            tc.tile_pool(name="small", bufs=2) as sm_pool,
            tc.tile_pool(name="s_ps", bufs=2, space=bass.MemorySpace.PSUM) as s_pool,
            tc.tile_pool(name="pv_ps", bufs=2, space=bass.MemorySpace.PSUM) as pv_pool,
            tc.tile_pool(name="mm_ps", bufs=2, space=bass.MemorySpace.PSUM) as mm_pool,
        ):
            # ---- persistent SBUF tensors (all bf16 matmul operands) ----
            x_sb = pp.tile([TT, NKT, S], BF16)       # [h%128, h//128, s] full x
            wq_sb = pp.tile([TT, NKT, DPC], BF16)
            wk_sb = pp.tile([TT, NKT, DPC], BF16)
            wv_sb = pp.tile([TT, NKT, DPC], BF16)
            wo_sb = pp.tile([TT, 2, HID], BF16)
            qT_sb = pp.tile([TT, 2, S], BF16)        # [d%128, d//128, s]
            kT_sb = pp.tile([TT, 2, S], BF16)
            v_sb = pp.tile([TT, NTT, HPC, HD + 1], BF16)  # [t%128, t//128, h, d|1]
            mask_sb = pp.tile([TT, TT], BF16)
            ones_sb = pp.tile([1, SC], BF16)
            if has_bias:
                bq_sb = pp.tile([1, DPC], BF16)
                bk_sb = pp.tile([1, DPC], BF16)

            # ---- all DMA loads up front; HWDGE serializes, so order by
            # need: weights q/k first, then x in 512-col chunks so pass 0
            # (which reads only cols [0:512)) starts after ~2 transfers.
            def load_w(dst, srcT, k0, k1, q):
                q.dma_start(
                    out=dst[:, k0:k1, :],
                    in_=srcT[TT * k0 : TT * k1, :].rearrange(
                        "(k p) d -> p k d", p=TT
                    ),
                )
            def load_x(k, c0, c1, q):
                q.dma_start(
                    out=x_sb[:, k, c0:c1], in_=xT[TT * k : TT * (k + 1), c0:c1]
                )
            def load_x4(k0, c0, c1, q):
                q.dma_start(
                    out=x_sb[:, k0 : k0 + 4, c0:c1],
                    in_=xT[TT * k0 : TT * (k0 + 4), c0:c1].rearrange(
                        "(k p) c -> p k c", p=TT
                    ),
                )
            load_w(wq_sb, wqT, 0, 2, nc.sync)
            load_x4(0, 0, SC, nc.sync)
            load_w(wk_sb, wkT, 0, 2, nc.scalar)
            load_w(wv_sb, wvT, 0, 2, nc.scalar)
            load_w(wq_sb, wqT, 2, 5, nc.sync)
            load_w(wk_sb, wkT, 2, 5, nc.scalar)
            load_x4(4, 0, SC, nc.sync)
            load_w(wv_sb, wvT, 2, 5, nc.scalar)
            load_w(wq_sb, wqT, 5, NKT, nc.sync)
            load_w(wk_sb, wkT, 5, NKT, nc.scalar)
            load_w(wv_sb, wvT, 5, NKT, nc.scalar)
            load_x4(0, SC, 2 * SC, nc.sync)
            load_x4(4, SC, 2 * SC, nc.scalar)
            nc.scalar.dma_start(out=mask_sb[:], in_=mskd)
            nc.scalar.dma_start(
                out=wo_sb[:], in_=woT.rearrange("(t p) d -> p t d", p=TT)
            )
            nc.sync.dma_start(
                out=x_sb[:, :, 2 * SC : 4 * SC],
                in_=xT[:, 2 * SC : 4 * SC].rearrange("(k p) c -> p k c", p=TT),
            )
            if has_bias:
                nc.scalar.dma_start(out=bq_sb[:], in_=bqr)
                nc.scalar.dma_start(out=bk_sb[:], in_=bkr)

            nc.vector.memset(ones_sb[:], 1.0)
            ones_c = sm_pool.tile([TT, NTT, HPC, 1], BF16, tag="ones_c")
            nc.vector.memset(ones_c[:], 1.0)
            # ones columns of the augmented v (softmax denominator trick)
            nc.vector.tensor_copy(v_sb[:, :, :, HD : HD + 1], ones_c[:])


            # ---- pass 0: k-streamed q/k (s-chunk 0) + v (t-tiles 0-3) ----
            # streams against the arriving per-k x DMAs so PE starts early.
            s_q = s_pool.tile([TT, 2, SC], F32, tag="s")
            s_k = s_pool.tile([TT, 2, SC], F32, tag="s")
            mmA = mm_pool.tile([TT, SC], F32, tag="mm")
            mmB = mm_pool.tile([TT, SC], F32, tag="mm")
            for k in range(NKT):
                st = k == 0
                sp = k == NKT - 1 and not has_bias
                for dti in range(2):
                    nc.tensor.matmul(
                        s_q[:, dti, :],
                        wq_sb[:, k, 128 * dti : 128 * (dti + 1)],
                        x_sb[:, k, 0:SC],
                        start=st, stop=sp,
                    )
                    nc.tensor.matmul(
                        s_k[:, dti, :],
                        wk_sb[:, k, 128 * dti : 128 * (dti + 1)],
                        x_sb[:, k, 0:SC],
                        start=st, stop=sp,
                    )

            def pass0_v():
                for k in range(NKT):
                    st = k == 0
                    for i in range(4):
                        dst = (mmA, mmB)[i // 2]
                        nc.tensor.matmul(
                            dst[:, 256 * (i % 2) : 256 * (i % 2) + 256],
                            x_sb[:, k, 128 * i : 128 * (i + 1)],
                            wv_sb[:, k, :],
                            start=(st and i % 2 == 0),
                            stop=(k == NKT - 1 and i % 2 == 1),
                        )
                nc.vector.tensor_copy(
                    v_sb[:, 0:2, :, 0:HD],
                    mmA[:].rearrange("p (t h d) -> p t h d", t=2, h=HPC),
                )
                nc.vector.tensor_copy(
                    v_sb[:, 2:4, :, 0:HD],
                    mmB[:].rearrange("p (t h d) -> p t h d", t=2, h=HPC),
                )
            if has_bias:
                for dti in range(2):
                    nc.tensor.matmul(
                        s_q[:, dti, :],
                        bq_sb[0:1, 128 * dti : 128 * (dti + 1)],
                        ones_sb[0:1, :], start=False, stop=True,
                    )
                    nc.tensor.matmul(
                        s_k[:, dti, :],
                        bk_sb[0:1, 128 * dti : 128 * (dti + 1)],
                        ones_sb[0:1, :], start=False, stop=True,
                    )
            nc.scalar.activation(kT_sb[:, :, 0:SC], s_k[:], COPY)
            nc.scalar.activation(qT_sb[:, :, 0:SC], s_q[:], COPY)

            # ---- remaining projection units (full-k accumulation each) ----
            def proj_qk(w_sb, b_sb, dst, dti, sc, eng):
                q_ps = mm_pool.tile([TT, SC], F32, tag="mm")
                for k in range(NKT):
                    nc.tensor.matmul(
                        q_ps[:],
                        w_sb[:, k, 128 * dti : 128 * (dti + 1)],
                        x_sb[:, k, SC * sc : SC * (sc + 1)],
                        start=(k == 0),
                        stop=(k == NKT - 1 and b_sb is None),
                    )
                if b_sb is not None:
                    nc.tensor.matmul(
                        q_ps[:],
                        b_sb[0:1, 128 * dti : 128 * (dti + 1)],
                        ones_sb[0:1, :], start=False, stop=True,
                    )
                if eng == "a":
                    nc.scalar.activation(
                        dst[:, dti, SC * sc : SC * (sc + 1)], q_ps[:], COPY
                    )
                else:
                    nc.vector.tensor_copy(
                        dst[:, dti, SC * sc : SC * (sc + 1)], q_ps[:]
                    )

            def proj_v(tt, eng):
                v_ps = mm_pool.tile([TT, SC], F32, tag="mm")
                for k in range(NKT):
                    nc.tensor.matmul(
                        v_ps[:, 0:DPC],
                        x_sb[:, k, 128 * tt : 128 * (tt + 1)],
                        wv_sb[:, k, :],
                        start=(k == 0),
                        stop=(k == NKT - 1),
                    )
                nc.vector.tensor_copy(
                    v_sb[:, tt, :, 0:HD],
                    v_ps[:, 0:DPC].rearrange("p (h d) -> p h d", h=HPC),
                )

            bq = bq_sb if has_bias else None
            bk = bk_sb if has_bias else None
            # passes 1-3: per pass sc: q(sc), k(sc), v(4 t-tiles). The last
            # pass's k/v units are deferred into the (ACT-bound) sc3 region:
            # each sc3 head consumes its diagonal kT/v tiles last.
            units = []
            late_units = []
            for sc in range(1, NSC):
                late = sc == NSC - 1
                qe = "a" if sc == 1 else "v"
                units.append(lambda sc=sc, e=qe: proj_qk(wq_sb, bq, qT_sb, 0, sc, e))
                units.append(lambda sc=sc: proj_qk(wq_sb, bq, qT_sb, 1, sc, "v"))
                (late_units if late else units).append(
                    lambda sc=sc, e=qe: proj_qk(wk_sb, bk, kT_sb, 0, sc, e)
                )
                (late_units if late else units).append(
                    lambda sc=sc: proj_qk(wk_sb, bk, kT_sb, 1, sc, "v")
                )
                for i in range(4):
                    (late_units if late else units).append(
                        lambda tt=4 * sc + i: proj_v(tt, "v")
                    )

            # ---- attention ----
            WAVE = 8

            def attn_qk(sc, h):
                # scores + exp + masks for every wave of this head; PV runs
                # later (software-pipelined one head behind) so the scalar
                # engine's exp stream never starves.
                dti, po = h // 2, 64 * (h % 2)
                n_tt = 4 * (sc + 1) if causal else NTT
                e_tiles = []
                for w0 in range(0, n_tt, WAVE):
                    wn = min(WAVE, n_tt - w0)
                    e_sb = e_pool.tile([TT, WAVE, SC], BF16)
                    e_tiles.append(e_sb)
                    for g0 in range(0, wn, 2):
                        s_ps = s_pool.tile([TT, 2, SC], F32, tag="s")
                        r0 = w0 + g0 - (n_tt - 4) if causal else -4
                        # diagonal pair (r2, r3): only columns [256:512) are
                        # ever read downstream -> compute/exp just those.
                        # Within a pair, tile r's PV later skips columns
                        # < 128*r entirely, so its QK can skip them too; the
                        # shared exp then reads stale-but-bounded psum there
                        # (old scores, |x|<64 -> exp finite) that nobody uses.
                        c0 = 256 if r0 == 2 else 0
                        for i in range(2):
                            tt = w0 + g0 + i
                            r = r0 + i
                            # tile r3's PV reads only cols [384:); its QK can
                            # skip [256:384) (separate exp below covers it)
                            qc0 = 384 if r == 3 else c0
                            nc.tensor.matmul(
                                s_ps[:, i, qc0:SC],
                                kT_sb[po : po + 64, dti,
                                      128 * tt : 128 * (tt + 1)],
                                qT_sb[po : po + 64, dti,
                                      SC * sc + qc0 : SC * (sc + 1)],
                                start=True,
                                stop=True,
                            )
                        if r0 == 2:
                            # (r2, r3): written column ranges differ -> two
                            # exps with exact ranges
                            nc.scalar.activation(
                                e_sb[:, g0, 256:SC], s_ps[:, 0, 256:SC],
                                EXP, scale=scale,
                            )
                            nc.scalar.activation(
                                e_sb[:, g0 + 1, 384:SC], s_ps[:, 1, 384:SC],
                                EXP, scale=scale,
                            )
                        else:
                            nc.scalar.activation(
                                e_sb[:, g0 : g0 + 2, c0:SC],
                                s_ps[:, :, c0:SC],
                                EXP,
                                scale=scale,
                            )
                        if r0 >= 0:
                            # triangular mask on each tile's own 128-col
                            # diagonal block
                            for i in range(2):
                                r = r0 + i
                                nc.vector.tensor_mul(
                                    e_sb[:, g0 + i, 128 * r : 128 * (r + 1)],
                                    e_sb[:, g0 + i, 128 * r : 128 * (r + 1)],
                                    mask_sb[:],
                                )
                return e_tiles

            def attn_pv(sc, h, e_tiles):
                # PV: outT_aug[65, s] += v_aug[t, 65].T @ expT[t, s]
                # (diagonal tiles skip their below-diagonal columns)
                n_tt = 4 * (sc + 1) if causal else NTT
                pv_ps = pv_pool.tile([HD + 1, SC], F32, tag="pv")
                for w0 in range(0, n_tt, WAVE):
                    wn = min(WAVE, n_tt - w0)
                    e_sb = e_tiles[w0 // WAVE]
                    for wi in range(wn):
                        tt = w0 + wi
                        r = tt - (n_tt - 4) if causal else -1
                        c0 = 128 * r if r > 0 else 0
                        nc.tensor.matmul(
                            pv_ps[:, c0:SC],
                            v_sb[:, tt, h, :],
                            e_sb[:, wi, c0:SC],
                            start=(tt == 0),
                            stop=(tt == n_tt - 1),
                        )
                return pv_ps

            def attn_norm(h, pv_ps, attn_sb, fast=False):
                # normalize: row 64 of pv_ps is the softmax denominator.
                # fast: split into two half-width chains on disjoint engine
                # sets (DVE+Pool vs DVE+PE+Pool) -- the last head's norm
                # gates the whole o_proj finale.
                dti, po = h // 2, 64 * (h % 2)
                if not fast:
                    rc_sb = sm_pool.tile([1, SC], F32, tag="rc")
                    nc.vector.reciprocal(rc_sb[:], pv_ps[64:65, :])
                    bc_sb = sm_pool.tile([HD, SC], F32, tag="bc")
                    nc.gpsimd.partition_broadcast(bc_sb[:], rc_sb[:], channels=HD)
                    nc.vector.tensor_mul(
                        attn_sb[po : po + 64, dti, :], pv_ps[0:64, :], bc_sb[:]
                    )
                    return
                w = SC // 2
                rcA = sm_pool.tile([1, SC], F32, tag="rc")
                nc.vector.reciprocal(rcA[0:1, 0:w], pv_ps[64:65, 0:w])
                rcB = sm_pool.tile([1, SC], F32, tag="rcb")
                nc.vector.reciprocal(rcB[0:1, 0:w], pv_ps[64:65, w:SC])
                bcA = sm_pool.tile([HD, SC], F32, tag="bc")
                nc.gpsimd.partition_broadcast(
                    bcA[:, 0:w], rcA[0:1, 0:w], channels=HD
                )
                bcB = sm_pool.tile([HD, SC], F32, tag="bcb")
                nc.gpsimd.partition_broadcast(
                    bcB[:, 0:w], rcB[0:1, 0:w], channels=HD
                )
                nc.vector.tensor_mul(
                    attn_sb[po : po + 64, dti, 0:w], pv_ps[0:64, 0:w],
                    bcA[:, 0:w],
                )
                nc.vector.tensor_mul(
                    attn_sb[po : po + 64, dti, w:SC], pv_ps[0:64, w:SC],
                    bcB[:, 0:w],
                )

            def oproj_unit(attn_sb, sc, et, eng):
                o_ps = mm_pool.tile([TT, SC], F32, tag="mm")
                for dti in range(2):
                    nc.tensor.matmul(
                        o_ps[:],
                        wo_sb[:, dti, 128 * et : 128 * (et + 1)],
                        attn_sb[:, dti, :],
                        start=(dti == 0),
                        stop=(dti == 1),
                    )
                o_sb = o_pool.tile([TT, SC], BF16)
                if eng == "a":
                    nc.scalar.activation(o_sb[:], o_ps[:], COPY)
                else:
                    nc.vector.tensor_copy(o_sb[:], o_ps[:])
                nc.sync.dma_start(
                    out=outT[128 * et : 128 * (et + 1), SC * sc : SC * (sc + 1)],
                    in_=o_sb[:],
                )

            fin_ps = {}

            def fin_mm0(attn_sb, et, pool=None, tag="mm", pair=False):
                if pair:
                    # two chunks share one 2-bank score-ring slot
                    o2 = (pool or s_pool).tile(
                        [TT, 2, SC], F32, tag=tag, name=f"fin{et}"
                    )
                    tiles = [o2[:, 0, :], o2[:, 1, :]]
                else:
                    t1 = (pool or mm_pool).tile(
                        [TT, SC], F32, tag=tag, name=f"fin{et}"
                    )
                    tiles = [t1[:]]
                for i, t in enumerate(tiles):
                    fin_ps[et + i] = t
                    nc.tensor.matmul(
                        t,
                        wo_sb[:, 0, 128 * (et + i) : 128 * (et + i + 1)],
                        attn_sb[:, 0, :], start=True, stop=False,
                    )

            def fin_mm1a(attn_sb, et):
                # first half-columns only need the first norm half (mulA)
                nc.tensor.matmul(
                    fin_ps[et][:, 0 : SC // 2],
                    wo_sb[:, 1, 128 * et : 128 * (et + 1)],
                    attn_sb[:, 1, 0 : SC // 2], start=False, stop=False,
                )

            def fin_mm1b(attn_sb, et):
                nc.tensor.matmul(
                    fin_ps[et][:, SC // 2 : SC],
                    wo_sb[:, 1, 128 * et : 128 * (et + 1)],
                    attn_sb[:, 1, SC // 2 : SC], start=False, stop=True,
                )

            def fin_rest(attn_sb, sc, et, o2, oi, eng):
                o_ps = fin_ps.pop(et)
                if eng == "v":
                    nc.vector.tensor_copy(o2[:, oi, :], o_ps)
                else:
                    nc.scalar.activation(o2[:, oi, :], o_ps, COPY)
                if oi == 1:
                    nc.sync.dma_start(
                        out=outT[128 * (et - 1) : 128 * (et + 1),
                                 SC * sc : SC * (sc + 1)].rearrange(
                            "(a p) n -> p a n", p=TT
                        ),
                        in_=o2[:],
                    )

            # deferred o_proj: sc's o_proj units run during sc+1's attention
            pending_o = []   # (attn_sb, sc, et, eng) chunks not yet emitted
            ui = 0
            OENG = ("a", "v", "a", "v", "a", "v", "v", "v")
            # late_units = [k3d0, k3d1, v12, v13, v14, v15]; in sc3 the
            # diagonal kT unit must precede the head's final-wave QK and the
            # v units its final-wave PV -- park them in those exp-bound gaps.
            for sc in range(NSC):
                last = sc == NSC - 1
                attn_sb = attn_pool.tile([TT, 2, SC], BF16)
                ets, pvs = {}, {}
                if last:
                    late_units[0]()     # kT tc3 dti0: before h0's diagonal QK
                for h in range(HPC):
                    if last and h == 2:
                        late_units[1]()  # kT tc3 dti1: before h2's QK
                    if last and h == HPC - 1:
                        # the finale's early dti0 matmuls (fin_mm0) hold the
                        # mm ring; drain leftover o chunks first
                        while pending_o:
                            oproj_unit(*pending_o.pop(0))
                    ets[h] = attn_qk(sc, h)
                    if h >= 1:
                        if sc == 0 and h == 1:
                            pass0_v()
                        if last and h == 1:
                            for u in late_units[2:6]:
                                u()      # v t12-15: before h0's diagonal PV
                        pvs[h - 1] = attn_pv(sc, h - 1, ets.pop(h - 1))
                    # fill PE bubbles: proj units first, then deferred o_proj
                    for _ in range(2):
                        if not last and ui < len(units):
                            units[ui]()
                            ui += 1
                        elif pending_o:
                            oproj_unit(*pending_o.pop(0))
                    if h >= 2:
                        attn_norm(h - 2, pvs.pop(h - 2), attn_sb)
                pvs[HPC - 1] = attn_pv(sc, HPC - 1, ets.pop(HPC - 1))
                if last:
                    fin_mm0(attn_sb, 0)
                    fin_mm0(attn_sb, 1)
                    fin_mm0(attn_sb, 2, s_pool, "s", pair=True)
                    fin_mm0(attn_sb, 4, s_pool, "s", pair=True)
                attn_norm(HPC - 2, pvs.pop(HPC - 2), attn_sb)
                attn_norm(HPC - 1, pvs.pop(HPC - 1), attn_sb, fast=True)
                while pending_o:
                    oproj_unit(*pending_o.pop(0))
                if last:
                    # last s-chunk: no later attention to hide behind. The
                    # dti0 halves of chunks 0-6 were issued during the last
                    # head's exp window / norm chain (the dti0 operand was
                    # normalized two heads ago), spread over the now-idle
                    # score/pv psum rings; only their dti1 halves + chunk 7
                    # remain after the last norm completes.
                    for et in range(NKT - 2):
                        fin_mm1a(attn_sb, et)
                    for et in range(NKT - 2):
                        fin_mm1b(attn_sb, et)
                    for et in range(NKT - 2):
                        if et % 2 == 0:
                            o2 = o_pool.tile([TT, 2, SC], BF16, name="o2")
                        fin_rest(attn_sb, sc, et, o2, et % 2,
                                 ("v", "g")[et % 2])
                    o2 = o_pool.tile([TT, 2, SC], BF16, name="o2l")
                    for et in range(NKT - 2, NKT):
                        o_ps = pv_pool.tile([TT, SC], F32, tag="pv",
                                            name=f"fino{et}")
                        for dti in range(2):
                            nc.tensor.matmul(
                                o_ps[:],
                                wo_sb[:, dti, 128 * et : 128 * (et + 1)],
                                attn_sb[:, dti, :],
                                start=(dti == 0),
                                stop=(dti == 1),
                            )
                        oi = et - (NKT - 2)
                        if et % 2 == 0:
                            nc.scalar.activation(o2[:, oi, :], o_ps[:], COPY)
                        else:
                            nc.vector.tensor_copy(o2[:, oi, :], o_ps[:])
                        nc.sync.dma_start(
                            out=outT[128 * et : 128 * (et + 1),
                                     SC * sc : SC * (sc + 1)],
                            in_=o2[:, oi, :],
                        )
                else:
                    eng = OENG if sc == 0 else ("v",) * NKT
                    pending_o = [(attn_sb, sc, et, eng[et]) for et in range(NKT)]
            while ui < len(units):
                units[ui]()
                ui += 1
    ctx_lp.__exit__(None, None, None)
    nc.compile()
    return nc


_CACHE = {}
LAST_RESULTS = None


def _get_nc(causal: bool, has_bias: bool = False):
    key = (causal, has_bias)
    if key not in _CACHE:
        _CACHE[key] = _build(causal, has_bias)
    return _CACHE[key]


def _reference_host(hidden_state, attention_mask, wq, bq, wk, bk, wv, bv, wo, bo):
    """Exact numpy fallback for unexpected mask patterns."""
    x = hidden_state.astype(np.float64)
    q = (x @ wq.T.astype(np.float64) + bq).reshape(B, S, NH, HD).transpose(0, 2, 1, 3)
    k = (x @ wk.T.astype(np.float64) + bk).reshape(B, S, NH, HD).transpose(0, 2, 1, 3)
    v = (x @ wv.T.astype(np.float64) + bv).reshape(B, S, NH, HD).transpose(0, 2, 1, 3)
    sc = np.einsum("bhsd,bhtd->bhst", q, k) / np.sqrt(HD)
    sc = np.where(attention_mask, sc, -np.inf)
    sc -= sc.max(axis=-1, keepdims=True)
    e = np.exp(sc)
    p = e / e.sum(axis=-1, keepdims=True)
    o = np.einsum("bhst,bhtd->bhsd", p, v).transpose(0, 2, 1, 3).reshape(B, S, HID)
    return (o @ wo.T.astype(np.float64) + bo).astype(np.float32)


def kernel(hidden_state, attention_mask, wq, bq, wk, bk, wv, bv, wo, bo):
    global LAST_RESULTS
    hidden_state = np.asarray(hidden_state, dtype=np.float32)
    attention_mask = np.asarray(attention_mask, dtype=bool)
    wq, bq = np.asarray(wq, np.float32), np.asarray(bq, np.float32)
    wk, bk = np.asarray(wk, np.float32), np.asarray(bk, np.float32)
    wv, bv = np.asarray(wv, np.float32), np.asarray(bv, np.float32)
    wo, bo = np.asarray(wo, np.float32), np.asarray(bo, np.float32)

    tril = np.tril(np.ones((S, S), dtype=bool))
    if (attention_mask == tril).all():
        causal = True
    elif attention_mask.all():
        causal = False
    else:
        return _reference_host(
            hidden_state, attention_mask, wq, bq, wk, bk, wv, bv, wo, bo
        )

    has_bias = bool(np.any(bq) or np.any(bk))
    mask_tri = np.triu(np.ones((TT, TT), dtype=BF))
    in_maps = []
    for c in range(N_CORES):
        b, g = c // 4, c % 4
        r0 = DPC * g
        m = {
            "xT": np.ascontiguousarray(hidden_state[b].T.astype(BF)),
            "wqT": np.ascontiguousarray(wq[r0 : r0 + DPC].T.astype(BF)),
            "wkT": np.ascontiguousarray(wk[r0 : r0 + DPC].T.astype(BF)),
            "wvT": np.ascontiguousarray(wv[r0 : r0 + DPC].T.astype(BF)),
            "woT": np.ascontiguousarray(wo[:, r0 : r0 + DPC].T.astype(BF)),
            "mask_tri": mask_tri,
        }
        if has_bias:
            m["bq_r"] = np.ascontiguousarray(
                bq[r0 : r0 + DPC].reshape(1, DPC).astype(BF)
            )
            m["bk_r"] = np.ascontiguousarray(
                bk[r0 : r0 + DPC].reshape(1, DPC).astype(BF)
            )
        in_maps.append(m)

    nc = _get_nc(causal, has_bias)
    res = run_bass_kernel_spmd(nc, in_maps, list(range(N_CORES)))
    LAST_RESULTS = res

    out = np.zeros((B, S, HID), dtype=np.float32)
    for c in range(N_CORES):
        out[c // 4] += res.results[c]["outT"].astype(np.float32).T
    out += (bv @ wo.T + bo)[None, None, :]
    return out
